# revision 4
# baseline (speedup 1.0000x reference)
"""GCN layer (x @ W, then sparse scatter-add by edges, + bias) on 8 Trainium2
NeuronCores via Bass/Tile.

Strategy (1-D graph partition, per sharding hint):
  out = A_sparse @ (x @ W) + bias = (A_sparse @ x) @ W + bias
- Pad node count 40000 -> 40960 = 8 cores x 5120 rows; core c owns dest rows
  [5120c, 5120(c+1)) = 40 dest tiles of 128 rows. The SPMD program is
  identical on all 8 cores; only input data differs.
- Host: bucket edges by (dest tile, src half). Halves because dma_gather
  indices are int16: "lo" edges (col < 32768) gather from x directly, "hi"
  edges gather from the x[32768:] offset view with col-32768.
  Each bucket's edge list is padded to whole groups of 128 slots; group
  counts per tile index are the max over cores (keeps the program uniform).
- Device, per batch of 4 dest tiles: two dma_gather calls (lo, hi) pull all
  source rows into SBUF G[p, g*128:(g+1)*128] = x[col of slot (p,g)].
  Per 128-edge group g: DVE builds onehot[e, d] = (iota[d]==dest_e)*val_e in
  one fused tensor_scalar; PE accumulates accT[din, dest] via
  matmul(lhsT=G_g, rhs=onehot) into PSUM over the tile's groups.
  Then accT -> SBUF, PE: final[dest, dout] = accT^T @ W + ones^T @ bias,
  final -> staged out SBUF; one DMA stores [128, 40, 128] per core at the
  end; host transposes to row-major, concatenates cores, slices padding.
"""

import os

import numpy as np

import concourse.bacc as bacc
import concourse.tile as tile
from concourse import bass, mybir
from concourse.bass_utils import run_bass_kernel_spmd

P = 128                      # partitions / dest-tile rows / edge-group size
D = 128                      # feature dim (D_in == D_out == 128)
N_NODES = 40000
N_CORES = 8
ROWS_PER_CORE = 5120         # 40960 / 8
TILES_PER_CORE = 40          # 5120 / 128
BATCH = 4                    # dest tiles per gather pair
SPLIT = 32768                # int16 index limit for dma_gather
F32 = mybir.dt.float32
I16 = mybir.dt.int16

_prog_cache: dict = {}
last_results = None          # BassKernelResults of the most recent run


def _layout(klo, khi):
    """Compute all column offsets from per-tile lo/hi group counts."""
    n_batches = TILES_PER_CORE // BATCH
    t = np.arange(TILES_PER_CORE)
    b = t // BATCH
    kb_lo = klo.reshape(n_batches, BATCH).sum(1)          # lo groups per batch
    kb_hi = khi.reshape(n_batches, BATCH).sum(1)
    kb = kb_lo + kb_hi
    batch_goff = np.concatenate([[0], np.cumsum(kb)])      # metadata col offsets
    blo_off = np.concatenate([[0], np.cumsum(kb_lo)])      # lo idx-buffer group offs
    bhi_off = np.concatenate([[0], np.cumsum(kb_hi)])
    lo_pref = np.zeros(TILES_PER_CORE, np.int64)           # lo prefix within batch
    hi_pref = np.zeros(TILES_PER_CORE, np.int64)
    for i in range(1, BATCH):
        lo_pref[i::BATCH] = lo_pref[i - 1 :: BATCH] + klo[i - 1 :: BATCH]
        hi_pref[i::BATCH] = hi_pref[i - 1 :: BATCH] + khi[i - 1 :: BATCH]
    mcol_lo = batch_goff[b] + lo_pref                      # metadata col base, lo
    mcol_hi = batch_goff[b] + kb_lo[b] + hi_pref           # metadata col base, hi
    return dict(
        kb_lo=kb_lo, kb_hi=kb_hi, kb=kb, batch_goff=batch_goff,
        blo_off=blo_off, bhi_off=bhi_off, lo_pref=lo_pref, hi_pref=hi_pref,
        mcol_lo=mcol_lo, mcol_hi=mcol_hi,
        g_tot=int(batch_goff[-1]), glo_tot=int(blo_off[-1]),
        ghi_tot=int(bhi_off[-1]),
    )


def _prep_edges(adj_row, adj_col, adj_val):
    """Bucket edges by (core, dest tile, src half); emit slot arrays."""
    E = adj_row.shape[0]
    n_tiles = N_CORES * TILES_PER_CORE
    tile_g = (adj_row >> 7).astype(np.int64)          # rows < 40000 -> 0..312
    half = (adj_col >= SPLIT).astype(np.int64)
    bucket = tile_g * 2 + half                        # 0..639
    order = np.argsort(bucket, kind="stable")
    bk = bucket[order]
    counts = np.bincount(bucket, minlength=2 * n_tiles)
    starts = np.zeros(2 * n_tiles + 1, np.int64)
    np.cumsum(counts, out=starts[1:])
    within = np.arange(E, dtype=np.int64) - starts[bk]

    cnt = counts.reshape(N_CORES, TILES_PER_CORE, 2)
    klo = np.maximum((cnt[:, :, 0].max(0) + P - 1) // P, 1).astype(np.int64)
    khi = ((cnt[:, :, 1].max(0) + P - 1) // P).astype(np.int64)
    lay = _layout(klo, khi)

    c = (bk // 2) // TILES_PER_CORE
    t = (bk // 2) % TILES_PER_CORE
    h = bk % 2
    b = t // BATCH

    # metadata slot -> (p, g) in [128, g_tot]
    mcol = np.where(h == 0, lay["mcol_lo"][t], lay["mcol_hi"][t]) + within // P
    p = within % P
    gdest = np.zeros((N_CORES, P, lay["g_tot"]), np.float32)
    gval = np.zeros((N_CORES, P, lay["g_tot"]), np.float32)
    gdest[c, p, mcol] = (adj_row[order] & 127).astype(np.float32)
    gval[c, p, mcol] = adj_val[order]

    # gather idx buffers (int16, 16-wrapped, replicated to 8 Q7 core groups)
    s_in_gather = np.where(
        h == 0, lay["lo_pref"][t], lay["hi_pref"][t]
    ) * P + within                                      # slot within batch-gather
    jcol = np.where(
        h == 0, 8 * lay["blo_off"][b], 8 * lay["bhi_off"][b]
    ) + s_in_gather // 16
    q = s_in_gather % 16
    gilo = np.zeros((N_CORES, 16, 8 * max(lay["glo_tot"], 1)), np.int16)
    gihi = np.zeros((N_CORES, 16, 8 * max(lay["ghi_tot"], 1)), np.int16)
    lo_m = h == 0
    hi_m = ~lo_m
    colv = adj_col[order].astype(np.int64)
    gilo[c[lo_m], q[lo_m], jcol[lo_m]] = colv[lo_m].astype(np.int16)
    gihi[c[hi_m], q[hi_m], jcol[hi_m]] = (colv[hi_m] - SPLIT).astype(np.int16)
    gilo = np.tile(gilo, (1, 8, 1))                    # replicate to 128 parts
    gihi = np.tile(gihi, (1, 8, 1))
    return klo, khi, lay, gdest, gval, gilo, gihi


def _build(klo_key, khi_key):
    cache_key = (klo_key, khi_key)
    if cache_key in _prog_cache:
        return _prog_cache[cache_key]
    klo = np.asarray(klo_key, np.int64)
    khi = np.asarray(khi_key, np.int64)
    lay = _layout(klo, khi)
    g_tot = lay["g_tot"]
    n_batches = TILES_PER_CORE // BATCH
    kb_max = int(lay["kb"].max())

    nc = bacc.Bacc("TRN2", num_devices=N_CORES)
    x_d = nc.dram_tensor("x", [N_NODES, D], F32, kind="ExternalInput")
    gilo_d = nc.dram_tensor("gilo", [P, 8 * max(lay["glo_tot"], 1)], I16,
                            kind="ExternalInput")
    gihi_d = nc.dram_tensor("gihi", [P, 8 * max(lay["ghi_tot"], 1)], I16,
                            kind="ExternalInput")
    gdest_d = nc.dram_tensor("gdest", [P, g_tot], F32, kind="ExternalInput")
    gval_d = nc.dram_tensor("gval", [P, g_tot], F32, kind="ExternalInput")
    iota_d = nc.dram_tensor("iota", [P, P], F32, kind="ExternalInput")
    w_d = nc.dram_tensor("weight", [D, D], F32, kind="ExternalInput")
    bias_d = nc.dram_tensor("bias", [1, D], F32, kind="ExternalInput")
    out_d = nc.dram_tensor("out", [P, TILES_PER_CORE, D], F32,
                           kind="ExternalOutput")

    with tile.TileContext(nc) as tc:
        with (
            tc.tile_pool(name="meta", bufs=1) as meta_pool,
            tc.tile_pool(name="gat", bufs=2) as gpool,
            tc.tile_pool(name="oh", bufs=8) as ohpool,
            tc.tile_pool(name="cp", bufs=3) as cpool,
            tc.tile_pool(name="ost", bufs=1) as opool,
            tc.tile_pool(name="acc", bufs=4, space="PSUM") as apool,
            tc.tile_pool(name="fin", bufs=2, space="PSUM") as fpool,
        ):
            gilo_sb = meta_pool.tile([P, 8 * max(lay["glo_tot"], 1)], I16)
            nc.sync.dma_start(out=gilo_sb[:], in_=gilo_d.ap())
            gihi_sb = meta_pool.tile([P, 8 * max(lay["ghi_tot"], 1)], I16)
            nc.sync.dma_start(out=gihi_sb[:], in_=gihi_d.ap())
            gdest_sb = meta_pool.tile([P, g_tot], F32)
            nc.sync.dma_start(out=gdest_sb[:], in_=gdest_d.ap())
            gval_sb = meta_pool.tile([P, g_tot], F32)
            nc.sync.dma_start(out=gval_sb[:], in_=gval_d.ap())
            iota_sb = meta_pool.tile([P, P], F32)
            nc.sync.dma_start(out=iota_sb[:], in_=iota_d.ap())
            w_sb = meta_pool.tile([D, D], F32)
            nc.sync.dma_start(out=w_sb[:], in_=w_d.ap())
            bias_sb = meta_pool.tile([1, D], F32)
            nc.sync.dma_start(out=bias_sb[:], in_=bias_d.ap())
            ones_sb = meta_pool.tile([1, P], F32)
            nc.vector.memset(ones_sb[:], 1.0)
            out_sb = opool.tile([P, TILES_PER_CORE * D], F32)

            for b in range(n_batches):
                kb_lo = int(lay["kb_lo"][b])
                kb_hi = int(lay["kb_hi"][b])
                kb = kb_lo + kb_hi
                gat = gpool.tile([P, kb_max * D], F32, tag="gat")
                if kb_lo:
                    nc.gpsimd.dma_gather(
                        gat[:, : kb_lo * D].rearrange("p (k d) -> p k d", d=D),
                        x_d.ap(),
                        gilo_sb[:, 8 * int(lay["blo_off"][b]) :
                                8 * int(lay["blo_off"][b]) + 8 * kb_lo],
                        kb_lo * P,
                        kb_lo * P,
                        D,
                        single_packet=False,
                    )
                if kb_hi:
                    nc.gpsimd.dma_gather(
                        gat[:, kb_lo * D : kb * D].rearrange(
                            "p (k d) -> p k d", d=D),
                        x_d.ap()[SPLIT:, :],
                        gihi_sb[:, 8 * int(lay["bhi_off"][b]) :
                                8 * int(lay["bhi_off"][b]) + 8 * kb_hi],
                        kb_hi * P,
                        kb_hi * P,
                        D,
                        single_packet=False,
                    )
                for t in range(b * BATCH, (b + 1) * BATCH):
                    groups = list(range(int(lay["mcol_lo"][t]),
                                        int(lay["mcol_lo"][t]) + int(klo[t])))
                    groups += list(range(int(lay["mcol_hi"][t]),
                                         int(lay["mcol_hi"][t]) + int(khi[t])))
                    acc = apool.tile([P, P], F32)
                    for j, g in enumerate(groups):
                        gc = g - int(lay["batch_goff"][b])   # column in gat
                        oh = ohpool.tile([P, P], F32)
                        nc.vector.tensor_scalar(
                            out=oh[:],
                            in0=iota_sb[:],
                            scalar1=gdest_sb[:, g : g + 1],
                            scalar2=gval_sb[:, g : g + 1],
                            op0=mybir.AluOpType.is_equal,
                            op1=mybir.AluOpType.mult,
                        )
                        nc.tensor.matmul(
                            out=acc[:],
                            lhsT=gat[:, gc * D : (gc + 1) * D],
                            rhs=oh[:],
                            start=(j == 0),
                            stop=(j == len(groups) - 1),
                        )
                    accs = cpool.tile([P, P], F32)
                    nc.scalar.copy(out=accs[:], in_=acc[:])
                    fin = fpool.tile([P, P], F32)
                    nc.tensor.matmul(out=fin[:], lhsT=accs[:], rhs=w_sb[:],
                                     start=True, stop=False)
                    nc.tensor.matmul(out=fin[:], lhsT=ones_sb[:],
                                     rhs=bias_sb[:], start=False, stop=True)
                    nc.scalar.copy(out=out_sb[:, t * D : (t + 1) * D],
                                   in_=fin[:])

            nc.sync.dma_start(out=out_d.ap(), in_=out_sb[:])

    nc.compile()
    _prog_cache[cache_key] = nc
    return nc


def kernel(x, adj_row, adj_col, adj_val, weight, bias):
    global last_results
    x = np.ascontiguousarray(np.asarray(x, dtype=np.float32))
    adj_row = np.asarray(adj_row, dtype=np.int32)
    adj_col = np.asarray(adj_col, dtype=np.int32)
    adj_val = np.asarray(adj_val, dtype=np.float32)
    weight = np.ascontiguousarray(np.asarray(weight, dtype=np.float32))
    bias = np.ascontiguousarray(np.asarray(bias, dtype=np.float32))

    klo, khi, lay, gdest, gval, gilo, gihi = _prep_edges(
        adj_row, adj_col, adj_val)
    nc = _build(tuple(int(k) for k in klo), tuple(int(k) for k in khi))

    iota = np.ascontiguousarray(
        np.broadcast_to(np.arange(P, dtype=np.float32), (P, P))
    )
    in_maps = [
        {
            "x": x,
            "gilo": np.ascontiguousarray(gilo[c]),
            "gihi": np.ascontiguousarray(gihi[c]),
            "gdest": np.ascontiguousarray(gdest[c]),
            "gval": np.ascontiguousarray(gval[c]),
            "iota": iota,
            "weight": weight,
            "bias": bias,
        }
        for c in range(N_CORES)
    ]
    last_results = run_bass_kernel_spmd(
        nc, in_maps, core_ids=list(range(N_CORES)),
        trace=bool(os.environ.get("GCN_TRACE")),
    )
    outs = [
        np.transpose(r["out"], (1, 0, 2)).reshape(ROWS_PER_CORE, D)
        for r in last_results.results
    ]
    return np.concatenate(outs, axis=0)[:N_NODES]


# revision 6
# speedup vs baseline: 1.0245x; 1.0245x over previous
"""GCN layer (x @ W, then sparse scatter-add by edges, + bias) on 8 Trainium2
NeuronCores via Bass/Tile.

Strategy (1-D graph partition, per sharding hint):
  out = A_sparse @ (x @ W) + bias = (A_sparse @ x) @ W + bias
- Pad node count 40000 -> 40960 = 8 cores x 5120 rows; core c owns dest rows
  [5120c, 5120(c+1)) = 40 dest tiles of 128 rows. The SPMD program is
  identical on all 8 cores; only input data differs.
- Gather trick: random 512B-row gathers run ~60 GB/s/core on HW, but 1024B
  descriptors run ~3x faster. So view x as row PAIRS [20000, 256] and gather
  pair idx = col>>1 (also fits dma_gather's int16 indices directly). Edges
  are bucketed per (dest tile, col parity) so each 128-edge group has
  constant parity and the matmul just slices the correct 128-col half of the
  gathered pair at compile time.
- Host: pad each bucket's edge list to whole groups of 128 slots; group
  counts per tile index are the max over cores (keeps the program uniform).
- Device, per batch of BATCH dest tiles: one dma_gather pulls all pair rows
  into SBUF G[p, g*256 + parity*128 + :128] = x[col of slot (p,g)].
  Per group g: DVE builds onehot[e, d] = (iota[d]==dest_e)*val_e in one
  fused tensor_scalar; PE accumulates accT[din, dest] via
  matmul(lhsT=G-half, rhs=onehot) into PSUM over the tile's groups.
  Then accT -> SBUF, PE: final[dest, dout] = accT^T @ W + ones^T @ bias,
  final -> staged out SBUF; one DMA stores [128, 40, 128] per core at the
  end; host transposes to row-major, concatenates cores, slices padding.
"""

import os

import numpy as np

import concourse.bacc as bacc
import concourse.tile as tile
from concourse import bass, mybir
from concourse.bass_utils import run_bass_kernel_spmd

P = 128                      # partitions / dest-tile rows / edge-group size
D = 128                      # feature dim (D_in == D_out == 128)
N_NODES = 40000
N_PAIRS = N_NODES // 2       # x viewed as [20000, 256]
N_CORES = 8
ROWS_PER_CORE = 5120         # 40960 / 8
TILES_PER_CORE = 40          # 5120 / 128
BATCH = 2                    # dest tiles per gather (num_idxs <= 8192 cap)
F32 = mybir.dt.float32
I16 = mybir.dt.int16

_prog_cache: dict = {}
last_results = None          # BassKernelResults of the most recent run


def _layout(kev, kod):
    """Column offsets from per-tile even/odd-parity group counts."""
    n_batches = TILES_PER_CORE // BATCH
    t = np.arange(TILES_PER_CORE)
    b = t // BATCH
    kb = (kev + kod).reshape(n_batches, BATCH).sum(1)      # groups per batch
    batch_goff = np.concatenate([[0], np.cumsum(kb)])      # group col offsets
    # prefix of groups within the batch before tile t
    pref = np.zeros(TILES_PER_CORE, np.int64)
    ktot = kev + kod
    for i in range(1, BATCH):
        pref[i::BATCH] = pref[i - 1 :: BATCH] + ktot[i - 1 :: BATCH]
    mcol_ev = batch_goff[b] + pref                         # even groups first
    mcol_od = mcol_ev + kev                                # then odd groups
    return dict(kb=kb, batch_goff=batch_goff, mcol_ev=mcol_ev,
                mcol_od=mcol_od, g_tot=int(batch_goff[-1]))


def _prep_edges(adj_row, adj_col, adj_val):
    """Bucket edges by (core, dest tile, col parity); emit slot arrays."""
    E = adj_row.shape[0]
    n_tiles = N_CORES * TILES_PER_CORE
    tile_g = (adj_row >> 7).astype(np.int64)          # rows < 40000 -> 0..312
    par = (adj_col & 1).astype(np.int64)
    bucket = tile_g * 2 + par                         # 0..639
    order = np.argsort(bucket, kind="stable")
    bk = bucket[order]
    counts = np.bincount(bucket, minlength=2 * n_tiles)
    starts = np.zeros(2 * n_tiles + 1, np.int64)
    np.cumsum(counts, out=starts[1:])
    within = np.arange(E, dtype=np.int64) - starts[bk]

    cnt = counts.reshape(N_CORES, TILES_PER_CORE, 2)
    kev = np.maximum((cnt[:, :, 0].max(0) + P - 1) // P, 1).astype(np.int64)
    kod = ((cnt[:, :, 1].max(0) + P - 1) // P).astype(np.int64)
    lay = _layout(kev, kod)
    g_tot = lay["g_tot"]

    c = (bk // 2) // TILES_PER_CORE
    t = (bk // 2) % TILES_PER_CORE
    h = bk % 2
    b = t // BATCH

    # metadata slot -> (p, g) in [128, g_tot]
    mcol = np.where(h == 0, lay["mcol_ev"][t], lay["mcol_od"][t]) + within // P
    p = within % P
    gdest = np.zeros((N_CORES, P, g_tot), np.float32)
    gval = np.zeros((N_CORES, P, g_tot), np.float32)
    gdest[c, p, mcol] = (adj_row[order] & 127).astype(np.float32)
    gval[c, p, mcol] = adj_val[order]

    # gather idx buffer (int16 pair indices, 16-wrapped, replicated x8)
    s = (mcol - lay["batch_goff"][b]) * P + p          # slot within batch
    jcol = 8 * lay["batch_goff"][b] + s // 16
    q = s % 16
    gidx = np.zeros((N_CORES, 16, 8 * g_tot), np.int16)
    gidx[c, q, jcol] = (adj_col[order] >> 1).astype(np.int16)
    gidx = np.tile(gidx, (1, 8, 1))                    # replicate to 128 parts
    return kev, kod, lay, gdest, gval, gidx


def _prep(x, adj_row, adj_col, adj_val, weight, bias):
    """Host prep shared by kernel() and the benchmarks."""
    x = np.ascontiguousarray(np.asarray(x, dtype=np.float32))
    adj_row = np.asarray(adj_row, dtype=np.int32)
    adj_col = np.asarray(adj_col, dtype=np.int32)
    adj_val = np.asarray(adj_val, dtype=np.float32)
    weight = np.ascontiguousarray(np.asarray(weight, dtype=np.float32))
    bias = np.ascontiguousarray(np.asarray(bias, dtype=np.float32))

    kev, kod, lay, gdest, gval, gidx = _prep_edges(adj_row, adj_col, adj_val)
    x2 = x.reshape(N_PAIRS, 2 * D)
    iota = np.ascontiguousarray(
        np.broadcast_to(np.arange(P, dtype=np.float32), (P, P)))
    in_maps = [
        {
            "x2": x2,
            "gidx": np.ascontiguousarray(gidx[c]),
            "gdest": np.ascontiguousarray(gdest[c]),
            "gval": np.ascontiguousarray(gval[c]),
            "iota": iota,
            "weight": weight,
            "bias": bias,
        }
        for c in range(N_CORES)
    ]
    key = (tuple(int(k) for k in kev), tuple(int(k) for k in kod))
    return key, in_maps


def _build(kev_key, kod_key, repeat=1):
    cache_key = (kev_key, kod_key, repeat)
    if cache_key in _prog_cache:
        return _prog_cache[cache_key]
    kev = np.asarray(kev_key, np.int64)
    kod = np.asarray(kod_key, np.int64)
    lay = _layout(kev, kod)
    g_tot = lay["g_tot"]
    n_batches = TILES_PER_CORE // BATCH
    kb_max = int(lay["kb"].max())
    assert kb_max * P <= 8192, "num_idxs cap per dma_gather"

    nc = bacc.Bacc("TRN2", num_devices=N_CORES)
    x_d = nc.dram_tensor("x2", [N_PAIRS, 2 * D], F32, kind="ExternalInput")
    gidx_d = nc.dram_tensor("gidx", [P, 8 * g_tot], I16, kind="ExternalInput")
    gdest_d = nc.dram_tensor("gdest", [P, g_tot], F32, kind="ExternalInput")
    gval_d = nc.dram_tensor("gval", [P, g_tot], F32, kind="ExternalInput")
    iota_d = nc.dram_tensor("iota", [P, P], F32, kind="ExternalInput")
    w_d = nc.dram_tensor("weight", [D, D], F32, kind="ExternalInput")
    bias_d = nc.dram_tensor("bias", [1, D], F32, kind="ExternalInput")
    out_d = nc.dram_tensor("out", [P, TILES_PER_CORE, D], F32,
                           kind="ExternalOutput")

    with tile.TileContext(nc) as tc:
        with (
            tc.tile_pool(name="meta", bufs=1) as meta_pool,
            tc.tile_pool(name="gat", bufs=2) as gpool,
            tc.tile_pool(name="oh", bufs=8) as ohpool,
            tc.tile_pool(name="cp", bufs=3) as cpool,
            tc.tile_pool(name="ost", bufs=1) as opool,
            tc.tile_pool(name="acc", bufs=4, space="PSUM") as apool,
            tc.tile_pool(name="fin", bufs=2, space="PSUM") as fpool,
        ):
            gidx_sb = meta_pool.tile([P, 8 * g_tot], I16)
            nc.sync.dma_start(out=gidx_sb[:], in_=gidx_d.ap())
            gdest_sb = meta_pool.tile([P, g_tot], F32)
            nc.sync.dma_start(out=gdest_sb[:], in_=gdest_d.ap())
            gval_sb = meta_pool.tile([P, g_tot], F32)
            nc.sync.dma_start(out=gval_sb[:], in_=gval_d.ap())
            iota_sb = meta_pool.tile([P, P], F32)
            nc.sync.dma_start(out=iota_sb[:], in_=iota_d.ap())
            w_sb = meta_pool.tile([D, D], F32)
            nc.sync.dma_start(out=w_sb[:], in_=w_d.ap())
            bias_sb = meta_pool.tile([1, D], F32)
            nc.sync.dma_start(out=bias_sb[:], in_=bias_d.ap())
            ones_sb = meta_pool.tile([1, P], F32)
            nc.vector.memset(ones_sb[:], 1.0)
            out_sb = opool.tile([P, TILES_PER_CORE * D], F32)

            for _rep in range(repeat):
                for b in range(n_batches):
                    kb = int(lay["kb"][b])
                    g0 = int(lay["batch_goff"][b])
                    gat = gpool.tile([P, kb_max * 2 * D], F32, tag="gat")
                    nc.gpsimd.dma_gather(
                        gat[:, : kb * 2 * D].rearrange(
                            "p (k d) -> p k d", d=2 * D),
                        x_d.ap(),
                        gidx_sb[:, 8 * g0 : 8 * (g0 + kb)],
                        kb * P,
                        kb * P,
                        2 * D,
                        single_packet=False,
                    )
                    for t in range(b * BATCH, (b + 1) * BATCH):
                        n_ev = int(kev[t])
                        n_od = int(kod[t])
                        groups = [(int(lay["mcol_ev"][t]) + i, 0)
                                  for i in range(n_ev)]
                        groups += [(int(lay["mcol_od"][t]) + i, 1)
                                   for i in range(n_od)]
                        acc = apool.tile([P, P], F32)
                        for j, (g, pg) in enumerate(groups):
                            gc = g - g0                 # group col in gat
                            oh = ohpool.tile([P, P], F32)
                            nc.vector.tensor_scalar(
                                out=oh[:],
                                in0=iota_sb[:],
                                scalar1=gdest_sb[:, g : g + 1],
                                scalar2=gval_sb[:, g : g + 1],
                                op0=mybir.AluOpType.is_equal,
                                op1=mybir.AluOpType.mult,
                            )
                            off = gc * 2 * D + pg * D
                            nc.tensor.matmul(
                                out=acc[:],
                                lhsT=gat[:, off : off + D],
                                rhs=oh[:],
                                start=(j == 0),
                                stop=(j == len(groups) - 1),
                            )
                        accs = cpool.tile([P, P], F32)
                        nc.scalar.copy(out=accs[:], in_=acc[:])
                        fin = fpool.tile([P, P], F32)
                        nc.tensor.matmul(out=fin[:], lhsT=accs[:], rhs=w_sb[:],
                                         start=True, stop=False)
                        nc.tensor.matmul(out=fin[:], lhsT=ones_sb[:],
                                         rhs=bias_sb[:], start=False,
                                         stop=True)
                        nc.scalar.copy(out=out_sb[:, t * D : (t + 1) * D],
                                       in_=fin[:])

            nc.sync.dma_start(out=out_d.ap(), in_=out_sb[:])

    nc.compile()
    _prog_cache[cache_key] = nc
    return nc


def kernel(x, adj_row, adj_col, adj_val, weight, bias):
    global last_results
    key, in_maps = _prep(x, adj_row, adj_col, adj_val, weight, bias)
    nc = _build(*key)
    last_results = run_bass_kernel_spmd(
        nc, in_maps, core_ids=list(range(N_CORES)),
        trace=bool(os.environ.get("GCN_TRACE")),
    )
    outs = [
        np.transpose(r["out"], (1, 0, 2)).reshape(ROWS_PER_CORE, D)
        for r in last_results.results
    ]
    return np.concatenate(outs, axis=0)[:N_NODES]


# revision 9
# speedup vs baseline: 5413.7410x; 5284.0266x over previous
"""GCN layer (x @ W, then sparse scatter-add by edges, + bias) on 8 Trainium2
NeuronCores via Bass/Tile.

Strategy (1-D graph partition, per sharding hint):
  out = A_sparse @ (x @ W) + bias = (A_sparse @ x) @ W + bias
- Pad node count 40000 -> 40960 = 8 cores x 5120 rows; core c owns dest rows
  [5120c, 5120(c+1)) = 40 dest tiles of 128 rows. The SPMD program is
  identical on all 8 cores; only input data differs.
- Gather trick: random 512B-row gathers run ~60 GB/s/core on HW, but 1024B
  descriptors run ~3x faster. So view x as row PAIRS [20000, 256] and gather
  pair idx = col>>1 (also fits dma_gather's int16 indices directly). Edges
  are bucketed per (dest tile, col parity) so each 128-edge group has
  constant parity and the matmul just slices the correct 128-col half of the
  gathered pair at compile time.
- Host: pad each bucket's edge list to whole groups of 128 slots; group
  counts per tile index are the max over cores (keeps the program uniform).
- Device, per batch of BATCH dest tiles: one dma_gather pulls all pair rows
  into SBUF G[p, g*256 + parity*128 + :128] = x[col of slot (p,g)].
  Per group g: DVE builds onehot[e, d] = (iota[d]==dest_e)*val_e in one
  fused tensor_scalar; PE accumulates accT[din, dest] via
  matmul(lhsT=G-half, rhs=onehot) into PSUM over the tile's groups.
  Then accT -> SBUF, PE: final[dest, dout] = accT^T @ W + ones^T @ bias,
  final -> staged out SBUF; one DMA stores [128, 40, 128] per core at the
  end; host transposes to row-major, concatenates cores, slices padding.
"""

import os

import numpy as np

import concourse.bacc as bacc
import concourse.tile as tile
from concourse import bass, mybir
from concourse.bass_utils import run_bass_kernel_spmd

P = 128                      # partitions / dest-tile rows / edge-group size
D = 128                      # feature dim (D_in == D_out == 128)
N_NODES = 40000
N_PAIRS = N_NODES // 2       # x viewed as [20000, 256]
N_CORES = 8
ROWS_PER_CORE = 5120         # 40960 / 8
TILES_PER_CORE = 40          # 5120 / 128
BATCH = 2                    # dest tiles per gather (num_idxs <= 8192 cap)
F32 = mybir.dt.float32
I16 = mybir.dt.int16

_prog_cache: dict = {}
last_results = None          # BassKernelResults of the most recent run


def _layout(kev, kod):
    """Column offsets from per-tile even/odd-parity group counts."""
    n_batches = TILES_PER_CORE // BATCH
    t = np.arange(TILES_PER_CORE)
    b = t // BATCH
    kb = (kev + kod).reshape(n_batches, BATCH).sum(1)      # groups per batch
    batch_goff = np.concatenate([[0], np.cumsum(kb)])      # group col offsets
    # prefix of groups within the batch before tile t
    pref = np.zeros(TILES_PER_CORE, np.int64)
    ktot = kev + kod
    for i in range(1, BATCH):
        pref[i::BATCH] = pref[i - 1 :: BATCH] + ktot[i - 1 :: BATCH]
    mcol_ev = batch_goff[b] + pref                         # even groups first
    mcol_od = mcol_ev + kev                                # then odd groups
    return dict(kb=kb, batch_goff=batch_goff, mcol_ev=mcol_ev,
                mcol_od=mcol_od, g_tot=int(batch_goff[-1]))


def _prep_edges(adj_row, adj_col, adj_val):
    """Bucket edges by (core, dest tile, col parity); emit slot arrays."""
    E = adj_row.shape[0]
    n_tiles = N_CORES * TILES_PER_CORE
    tile_g = (adj_row >> 7).astype(np.int64)          # rows < 40000 -> 0..312
    par = (adj_col & 1).astype(np.int64)
    bucket = tile_g * 2 + par                         # 0..639
    order = np.argsort(bucket, kind="stable")
    bk = bucket[order]
    counts = np.bincount(bucket, minlength=2 * n_tiles)
    starts = np.zeros(2 * n_tiles + 1, np.int64)
    np.cumsum(counts, out=starts[1:])
    within = np.arange(E, dtype=np.int64) - starts[bk]

    cnt = counts.reshape(N_CORES, TILES_PER_CORE, 2)
    kev = np.maximum((cnt[:, :, 0].max(0) + P - 1) // P, 1).astype(np.int64)
    kod = ((cnt[:, :, 1].max(0) + P - 1) // P).astype(np.int64)
    lay = _layout(kev, kod)
    g_tot = lay["g_tot"]

    c = (bk // 2) // TILES_PER_CORE
    t = (bk // 2) % TILES_PER_CORE
    h = bk % 2
    b = t // BATCH

    # metadata slot -> (p, g) in [128, g_tot]
    mcol = np.where(h == 0, lay["mcol_ev"][t], lay["mcol_od"][t]) + within // P
    p = within % P
    gdest = np.zeros((N_CORES, P, g_tot), np.float32)
    gval = np.zeros((N_CORES, P, g_tot), np.float32)
    gdest[c, p, mcol] = (adj_row[order] & 127).astype(np.float32)
    gval[c, p, mcol] = adj_val[order]

    # gather idx buffer (int16 pair indices, 16-wrapped, replicated x8)
    s = (mcol - lay["batch_goff"][b]) * P + p          # slot within batch
    jcol = 8 * lay["batch_goff"][b] + s // 16
    q = s % 16
    gidx = np.zeros((N_CORES, 16, 8 * g_tot), np.int16)
    gidx[c, q, jcol] = (adj_col[order] >> 1).astype(np.int16)
    gidx = np.tile(gidx, (1, 8, 1))                    # replicate to 128 parts
    return kev, kod, lay, gdest, gval, gidx


def _prep(x, adj_row, adj_col, adj_val, weight, bias):
    """Host prep shared by kernel() and the benchmarks."""
    x = np.ascontiguousarray(np.asarray(x, dtype=np.float32))
    adj_row = np.asarray(adj_row, dtype=np.int32)
    adj_col = np.asarray(adj_col, dtype=np.int32)
    adj_val = np.asarray(adj_val, dtype=np.float32)
    weight = np.ascontiguousarray(np.asarray(weight, dtype=np.float32))
    bias = np.ascontiguousarray(np.asarray(bias, dtype=np.float32))

    kev, kod, lay, gdest, gval, gidx = _prep_edges(adj_row, adj_col, adj_val)
    x2 = x.reshape(N_PAIRS, 2 * D)
    iota = np.ascontiguousarray(
        np.broadcast_to(np.arange(P, dtype=np.float32), (P, P)))
    in_maps = [
        {
            "x2": x2,
            "gidx": np.ascontiguousarray(gidx[c]),
            "gdest": np.ascontiguousarray(gdest[c]),
            "gval": np.ascontiguousarray(gval[c]),
            "iota": iota,
            "weight": weight,
            "bias": bias,
        }
        for c in range(N_CORES)
    ]
    key = (tuple(int(k) for k in kev), tuple(int(k) for k in kod))
    return key, in_maps


def _build(kev_key, kod_key, repeat=1):
    cache_key = (kev_key, kod_key, repeat)
    if cache_key in _prog_cache:
        return _prog_cache[cache_key]
    kev = np.asarray(kev_key, np.int64)
    kod = np.asarray(kod_key, np.int64)
    lay = _layout(kev, kod)
    g_tot = lay["g_tot"]
    n_batches = TILES_PER_CORE // BATCH
    kb_max = int(lay["kb"].max())
    assert kb_max * P <= 8192, "num_idxs cap per dma_gather"

    nc = bacc.Bacc("TRN2", num_devices=N_CORES)
    x_d = nc.dram_tensor("x2", [N_PAIRS, 2 * D], F32, kind="ExternalInput")
    gidx_d = nc.dram_tensor("gidx", [P, 8 * g_tot], I16, kind="ExternalInput")
    gdest_d = nc.dram_tensor("gdest", [P, g_tot], F32, kind="ExternalInput")
    gval_d = nc.dram_tensor("gval", [P, g_tot], F32, kind="ExternalInput")
    iota_d = nc.dram_tensor("iota", [P, P], F32, kind="ExternalInput")
    w_d = nc.dram_tensor("weight", [D, D], F32, kind="ExternalInput")
    bias_d = nc.dram_tensor("bias", [1, D], F32, kind="ExternalInput")
    out_d = nc.dram_tensor("out", [P, TILES_PER_CORE, D], F32,
                           kind="ExternalOutput")

    with tile.TileContext(nc) as tc:
        with (
            tc.tile_pool(name="meta", bufs=1) as meta_pool,
            tc.tile_pool(name="gat", bufs=3) as gpool,
            tc.tile_pool(name="oh", bufs=8) as ohpool,
            tc.tile_pool(name="cp", bufs=3) as cpool,
            tc.tile_pool(name="ost", bufs=1) as opool,
            tc.tile_pool(name="acc", bufs=4, space="PSUM") as apool,
            tc.tile_pool(name="fin", bufs=2, space="PSUM") as fpool,
        ):
            gidx_sb = meta_pool.tile([P, 8 * g_tot], I16)
            nc.sync.dma_start(out=gidx_sb[:], in_=gidx_d.ap())
            gdest_sb = meta_pool.tile([P, g_tot], F32)
            nc.sync.dma_start(out=gdest_sb[:], in_=gdest_d.ap())
            gval_sb = meta_pool.tile([P, g_tot], F32)
            nc.sync.dma_start(out=gval_sb[:], in_=gval_d.ap())
            iota_sb = meta_pool.tile([P, P], F32)
            nc.sync.dma_start(out=iota_sb[:], in_=iota_d.ap())
            w_sb = meta_pool.tile([D, D], F32)
            nc.sync.dma_start(out=w_sb[:], in_=w_d.ap())
            bias_sb = meta_pool.tile([1, D], F32)
            nc.sync.dma_start(out=bias_sb[:], in_=bias_d.ap())
            ones_sb = meta_pool.tile([1, P], F32)
            nc.vector.memset(ones_sb[:], 1.0)
            out_sb = opool.tile([P, TILES_PER_CORE * D], F32)

            for _rep in range(repeat):
                for b in range(n_batches):
                    kb = int(lay["kb"][b])
                    g0 = int(lay["batch_goff"][b])
                    gat = gpool.tile([P, kb_max * 2 * D], F32, tag="gat")
                    nc.gpsimd.dma_gather(
                        gat[:, : kb * 2 * D].rearrange(
                            "p (k d) -> p k d", d=2 * D),
                        x_d.ap(),
                        gidx_sb[:, 8 * g0 : 8 * (g0 + kb)],
                        kb * P,
                        kb * P,
                        2 * D,
                        single_packet=False,
                    )
                    for t in range(b * BATCH, (b + 1) * BATCH):
                        n_ev = int(kev[t])
                        n_od = int(kod[t])
                        groups = [(int(lay["mcol_ev"][t]) + i, 0)
                                  for i in range(n_ev)]
                        groups += [(int(lay["mcol_od"][t]) + i, 1)
                                   for i in range(n_od)]
                        acc = apool.tile([P, P], F32)
                        for j, (g, pg) in enumerate(groups):
                            gc = g - g0                 # group col in gat
                            oh = ohpool.tile([P, P], F32)
                            nc.vector.tensor_scalar(
                                out=oh[:],
                                in0=iota_sb[:],
                                scalar1=gdest_sb[:, g : g + 1],
                                scalar2=gval_sb[:, g : g + 1],
                                op0=mybir.AluOpType.is_equal,
                                op1=mybir.AluOpType.mult,
                            )
                            off = gc * 2 * D + pg * D
                            nc.tensor.matmul(
                                out=acc[:],
                                lhsT=gat[:, off : off + D],
                                rhs=oh[:],
                                start=(j == 0),
                                stop=(j == len(groups) - 1),
                            )
                        accs = cpool.tile([P, P], F32)
                        nc.scalar.copy(out=accs[:], in_=acc[:])
                        fin = fpool.tile([P, P], F32)
                        nc.tensor.matmul(out=fin[:], lhsT=accs[:], rhs=w_sb[:],
                                         start=True, stop=False)
                        nc.tensor.matmul(out=fin[:], lhsT=ones_sb[:],
                                         rhs=bias_sb[:], start=False,
                                         stop=True)
                        nc.scalar.copy(out=out_sb[:, t * D : (t + 1) * D],
                                       in_=fin[:])

            nc.sync.dma_start(out=out_d.ap(), in_=out_sb[:])

    nc.compile()
    _prog_cache[cache_key] = nc
    return nc


def kernel(x, adj_row, adj_col, adj_val, weight, bias):
    global last_results
    key, in_maps = _prep(x, adj_row, adj_col, adj_val, weight, bias)
    nc = _build(*key)
    last_results = run_bass_kernel_spmd(
        nc, in_maps, core_ids=list(range(N_CORES)),
        trace=bool(os.environ.get("GCN_TRACE")),
    )
    outs = [
        np.transpose(r["out"], (1, 0, 2)).reshape(ROWS_PER_CORE, D)
        for r in last_results.results
    ]
    return np.concatenate(outs, axis=0)[:N_NODES]
